# revision 3
# baseline (speedup 1.0000x reference)
"""Trainium2 Bass kernel for nn_Critic (MLP value function + GAE).

Sharding: batch B=2048 split across 8 NeuronCores (256 each). MLP params
replicated. The time recurrence (reverse GAE scan) is independent per batch
element, so no cross-core communication.

Strategy (single-pass bf16; PE streaming floor ~464 us/core):
  - Host pre-transposes states to [P, KD, T+1 * BC] bf16 per core, so the PE
    does zero transposes; DMA loads feature-major k-tiles directly.
  - Tokens (t, b) are flattened: 17*256 = 4352 tokens per core, processed
    in chunks of 512 (one fp32 PSUM bank). All matmuls single-pass bf16
    (1 col/cycle @2.4GHz warm): end-to-end max relerr ~5e-3 vs the 2e-2 gate.
  - Startup (the trace showed first MATMUL at 14.5us with DMAs issued
    serially on Sync at ~700ns each): chunk-0 state tiles + W0 pieces are
    issued from FOUR engines in parallel (sync/scalar/vector/gpsimd), W0's
    m=0 block is split so the first LDWEIGHTS only waits on a 128KB slice,
    and 8 dummy matmuls on a memset tile warm the PE HAM clock (cold 1.2GHz
    -> warm 2.4GHz needs ~3.4us of busy) while a dummy Exp preloads the
    ScalarE activation table (~2.7us) before the first real ELU.
  - Steady state: ONE 3D-AP DMA per chunk loads all 16 k-tiles of states
    (144 -> 8 dma_starts; each dma_start costs ~700ns of issue and the
    kernel-tail semaphore teardown scales with sync traffic). W1/W2/biases/
    GAE inputs are single consolidated DMAs.
  - ELU(z) = min(exp(z)-1, relu(z)): ScalarE Exp + ScalarE Relu (both with
    fused +bias from PSUM), one VectorE combine writing bf16 directly.
  - value head: h3 (bf16) stationary [128 h, 128 tokens], Wo column moving
    -> psum [128 tokens, 1] accumulated over 8 k-tiles. Head matmuls for
    chunk c are deferred into chunk c+1's layer-0 stream so the PE never
    waits on the last ELU at a chunk boundary. ScalarE Identity with fused
    +bo writes valT [128 batch, 17 time] (stored time-reversed). The LAST
    chunk's head is pipelined into its own L2 m-loop (head k-tile m-2
    after m's ELU) so only k=6,7 trail the final ELU.
  - GAE: a handful of [128, 16/17] VectorE ops; the reverse scan is a
    single tensor_tensor_scan (state = dl*state + delta) since the host
    pre-reverses reward/cont and valT is written reversed. delta columns
    1..15 are precomputed before the final head lands so only column 0
    (t=16) trails it.
"""

import sys

sys.path.insert(0, "/opt/trn_rl_repo")

import numpy as np

T, B, D, H = 16, 2048, 2048, 1024
NCORES = 8
BC = B // NCORES  # 256 batch per core
TP1 = T + 1
TOK = TP1 * BC  # 4352 tokens per core
DISCOUNT, LAMBDA = 0.99, 0.95
P = 128
KD = D // P  # 16 k-tiles for layer 0
KH = H // P  # 8 k-tiles for layers 1,2,out
MH = H // P  # 8 m-tiles of hidden units
CH = 512  # tokens per chunk (one PSUM bank of fp32)
NCH = (TOK + CH - 1) // CH  # 9 chunks: 8 full + 1 of 256
NWARM = 8  # dummy matmuls to warm the PE HAM clock gate

_NC_CACHE = None


def _build():
    import concourse.bacc as bacc
    import concourse.mybir as mybir
    from concourse.tile import TileContext

    F32 = mybir.dt.float32
    BF16 = mybir.dt.bfloat16
    ALU = mybir.AluOpType
    ACTF = mybir.ActivationFunctionType

    nc = bacc.Bacc(None, target_bir_lowering=False, debug=False)

    # states as [p, k, tok]: one 3D-AP DMA per chunk covers all 16 k-tiles
    st_h = nc.declare_dram_parameter("statesP", [P, KD, TOK], BF16, isOutput=False)
    # W0 host-reordered m-major: row m*P+p, col k*P+q  <-  W0[k*P+p, m*P+q],
    # so each output-column block m loads with ONE efficient DMA (4KB rows)
    w0_h = nc.declare_dram_parameter("W0", [MH * P, KD * P], BF16, isOutput=False)
    # W1/W2 as [p, k, col]: one DMA each
    w1_h = nc.declare_dram_parameter("W1", [P, KH, H], BF16, isOutput=False)
    w2_h = nc.declare_dram_parameter("W2", [P, KH, H], BF16, isOutput=False)
    wo_h = nc.declare_dram_parameter("Wo", [P, KH], BF16, isOutput=False)
    bo_h = nc.declare_dram_parameter("bo_b", [P, 1], F32, isOutput=False)
    # b0|b1|b2 packed: one DMA
    bias_h = nc.declare_dram_parameter("biases", [P, 3 * MH], F32, isOutput=False)
    # cont_rev|rew_rev packed per 128-batch block: one DMA each
    gae_h = nc.declare_dram_parameter("gae_in", [BC, TP1 + T], F32, isOutput=False)
    ret_h = nc.declare_dram_parameter("ret_bt", [BC, T], F32, isOutput=True)
    val_h = nc.declare_dram_parameter("val_bt", [BC, T], F32, isOutput=True)

    with TileContext(nc) as tc:
        with (
            tc.tile_pool(name="warm", bufs=1) as warmpool,
            tc.tile_pool(name="wpool", bufs=1) as wpool,
            tc.tile_pool(name="x0pool", bufs=1) as x0pool,
            tc.tile_pool(name="xpool", bufs=2) as xpool,
            tc.tile_pool(name="hpool", bufs=2) as hpool,
            tc.tile_pool(name="tmp", bufs=4) as tmppool,
            tc.tile_pool(name="gae", bufs=1) as gaepool,
            tc.tile_pool(name="psA", bufs=6, space="PSUM") as psApool,
            tc.tile_pool(name="psV", bufs=2, space="PSUM") as psVpool,
        ):
            # ---- PE/ACT warmup: memset tile -> 8 dummy matmuls + dummy Exp.
            # No DMA deps, so these issue immediately (~6.3us, engine bring-up
            # done) and run while the first input DMAs are in flight; the HAM
            # clock gate un-throttles after ~3.4us of sustained PE activity.
            warm = warmpool.tile([P, CH], BF16, name="warm", tag="warm")
            nc.gpsimd.memset(warm[:], 0.0)
            wtmp = warmpool.tile([P, 1], F32, name="wtmp", tag="wtmp")
            for _ in range(NWARM):
                psd = psApool.tile([P, CH], F32, name="ps", tag="ps")
                nc.tensor.matmul(
                    psd[:],
                    lhsT=warm[:, 0:P],
                    rhs=warm[:],
                    start=True,
                    stop=True,
                    skip_group_check=True,
                )

            # ---- weight / input tiles ----
            w0m0a = wpool.tile([P, 4 * P], BF16, name="w0m0a", tag="w0m0a")
            w0m0b = wpool.tile([P, (KD - 4) * P], BF16, name="w0m0b", tag="w0m0b")
            w0m = [None] + [
                wpool.tile([P, KD * P], BF16, name=f"w0m{m}", tag=f"w0m{m}")
                for m in range(1, MH)
            ]
            w1sb = wpool.tile([P, KH, H], BF16, name="w1sb", tag="w1sb")
            w2sb = wpool.tile([P, KH, H], BF16, name="w2sb", tag="w2sb")
            wosb = wpool.tile([P, KH], BF16, name="wosb", tag="wosb")
            bosb = wpool.tile([P, 1], F32, name="bosb", tag="bosb")
            biassb = wpool.tile([P, 3 * MH], F32, name="biassb", tag="biassb")
            xts0 = [
                x0pool.tile([P, CH], BF16, name=f"xt{k}", tag=f"xt{k}")
                for k in range(KD)
            ]
            gaesb = []
            valT = []
            for blk in range(2):
                g = gaepool.tile(
                    [P, TP1 + T], F32, name=f"gaesb{blk}", tag=f"gaesb{blk}"
                )
                gaesb.append(g)
                vt = gaepool.tile([P, TP1], F32, name=f"valT{blk}", tag=f"valT{blk}")
                valT.append(vt)
            contsb = [g[:, 0:TP1] for g in gaesb]
            rewsb = [g[:, TP1 : TP1 + T] for g in gaesb]

            # ---- startup DMAs, issued from THREE engines in parallel (only
            # sync/scalar/gpsimd can trigger DMAs). Per-engine program order
            # = issue order; each dma_start costs ~700ns of that engine's
            # time. First-MM deps (xt0 + w0m0a) go first on their queues.
            # scalar: w0 m=0 (split), dummy-Exp table preload, w0 m=1..5,
            # biases, wo — done by ~14.5us, before the first real ELU.
            nc.scalar.dma_start(out=w0m0a[:], in_=w0_h[0:P, 0 : 4 * P])
            nc.scalar.dma_start(out=w0m0b[:], in_=w0_h[0:P, 4 * P : KD * P])
            nc.scalar.activation(wtmp[:], warm[:, 0:1], ACTF.Exp)
            nc.scalar.dma_start(out=w0m[1][:], in_=w0_h[P : 2 * P, :])
            nc.scalar.dma_start(out=biassb[:], in_=bias_h[:])
            for m in range(2, 6):
                nc.scalar.dma_start(out=w0m[m][:], in_=w0_h[m * P : (m + 1) * P, :])
            nc.scalar.dma_start(out=wosb[:], in_=wo_h[:])
            # sync / gpsimd: chunk-0 state k-tiles round-robin
            for k in range(KD):
                eng = (nc.sync, nc.gpsimd)[k % 2]
                eng.dma_start(out=xts0[k][:], in_=st_h[:, k, 0:CH])
            # gpsimd: rest of W0, W1/W2, bo, GAE inputs (all needed late)
            for m in range(6, MH):
                nc.gpsimd.dma_start(out=w0m[m][:], in_=w0_h[m * P : (m + 1) * P, :])
            nc.gpsimd.dma_start(out=w1sb[:], in_=w1_h[:])
            nc.gpsimd.dma_start(out=w2sb[:], in_=w2_h[:])
            nc.gpsimd.dma_start(out=bosb[:], in_=bo_h[:])
            for blk in range(2):
                nc.gpsimd.dma_start(
                    out=gaesb[blk][:], in_=gae_h[blk * P : (blk + 1) * P, :]
                )

            def l0_lhsT(k, m):
                if m == 0:
                    if k < 4:
                        return w0m0a[:, k * P : (k + 1) * P]
                    return w0m0b[:, (k - 4) * P : (k - 3) * P]
                return w0m[m][:, k * P : (k + 1) * P]

            # ---- chunked fused MLP over flattened (t, b) tokens ----
            def layer(lhsT_of_km, nk, rhs_of_k, bias_col, houts, n, after_m=()):
                for m in range(MH):
                    ps = psApool.tile([P, CH], F32, name="ps", tag="ps")
                    for k in range(nk):
                        nc.tensor.matmul(
                            ps[:, :n],
                            lhsT=lhsT_of_km(k, m),
                            rhs=rhs_of_k(k),
                            start=(k == 0),
                            stop=(k == nk - 1),
                            skip_group_check=True,
                        )
                    e = tmppool.tile([P, CH], F32, name="e", tag="e")
                    nc.scalar.activation(
                        e[:, :n],
                        ps[:, :n],
                        ACTF.Exp,
                        bias=biassb[:, bias_col + m : bias_col + m + 1],
                    )
                    rl = tmppool.tile([P, CH], F32, name="rl", tag="rl")
                    nc.scalar.activation(
                        rl[:, :n],
                        ps[:, :n],
                        ACTF.Relu,
                        bias=biassb[:, bias_col + m : bias_col + m + 1],
                    )
                    nc.vector.scalar_tensor_tensor(
                        houts[:, m, 0:n],
                        e[:, :n],
                        1.0,
                        rl[:, :n],
                        ALU.subtract,
                        ALU.min,
                    )
                    if m < len(after_m) and after_m[m] is not None:
                        after_m[m]()

            # value head for one 128-token block: h3 stationary, Wo moving.
            def make_head(c, h3, tb):
                g = c * (CH // P) + tb  # global 128-token block
                t_idx = g // 2
                blk = g % 2

                def emit():
                    pv = psVpool.tile([P, 1], F32, name="pv", tag="pv")
                    for k in range(KH):
                        nc.tensor.matmul(
                            pv[:],
                            lhsT=h3[:, k, tb * P : tb * P + P],
                            rhs=wosb[:, k : k + 1],
                            start=(k == 0),
                            stop=(k == KH - 1),
                            skip_group_check=True,
                        )
                    # store time-REVERSED: column 16-t, with fused +bo
                    nc.scalar.activation(
                        valT[blk][:, TP1 - 1 - t_idx : TP1 - t_idx],
                        pv[:],
                        ACTF.Identity,
                        bias=bosb[:],
                    )

                return emit

            dls = [None, None]
            dtts = [None, None]

            pending_head = []
            for c in range(NCH):
                base = c * CH
                n = min(CH, TOK - base)
                last = c == NCH - 1
                if c == 0:
                    xts = None
                else:
                    xts = xpool.tile([P, KD, CH], BF16, name="xts", tag="xts")
                    nc.sync.dma_start(out=xts[:, :, 0:n], in_=st_h[:, :, base : base + n])

                h1 = hpool.tile([P, MH, CH], BF16, name="h1", tag="h1")
                layer(
                    l0_lhsT,
                    KD,
                    (lambda k: xts0[k][:, :n]) if c == 0 else (lambda k: xts[:, k, 0:n]),
                    0,
                    h1,
                    n,
                    after_m=pending_head,
                )
                pending_head = []

                if c == 1:
                    # dl = cont*DISCOUNT*LAMBDA for the scan; Vector is
                    # otherwise idle and the GAE inputs landed at startup.
                    for blk in range(2):
                        dl = gaepool.tile([P, T], F32, name=f"dl{blk}", tag=f"dl{blk}")
                        nc.vector.tensor_scalar_mul(
                            dl[:], contsb[blk][:, 0:T], DISCOUNT * LAMBDA
                        )
                        dls[blk] = dl
                if last:
                    # deltas for time-reversed columns 1..15 depend only on
                    # value columns 1..16, all of which landed with chunk 7's
                    # deferred head (played in this chunk's L0). Precompute
                    # them so only column 0 (t=16) trails the final head.
                    for blk in range(2):
                        dtt = gaepool.tile(
                            [P, T], F32, name=f"dtt{blk}", tag=f"dtt{blk}"
                        )
                        nc.vector.scalar_tensor_tensor(
                            dtt[:, 1:T],
                            contsb[blk][:, 1:T],
                            DISCOUNT,
                            valT[blk][:, 1:T],
                            ALU.mult,
                            ALU.mult,
                        )
                        nc.vector.tensor_add(
                            dtt[:, 1:T], dtt[:, 1:T], rewsb[blk][:, 1:T]
                        )
                        nc.vector.tensor_sub(
                            dtt[:, 1:T], dtt[:, 1:T], valT[blk][:, 2:TP1]
                        )
                        dtts[blk] = dtt

                h2 = hpool.tile([P, MH, CH], BF16, name="h2", tag="h2")
                layer(
                    lambda k, m: w1sb[:, k, m * P : (m + 1) * P],
                    KH,
                    lambda k: h1[:, k, 0:n],
                    MH,
                    h2,
                    n,
                )
                h3 = hpool.tile([P, MH, CH], BF16, name="h3", tag="h3")

                if not last:
                    layer(
                        lambda k, m: w2sb[:, k, m * P : (m + 1) * P],
                        KH,
                        lambda k: h2[:, k, 0:n],
                        2 * MH,
                        h3,
                        n,
                    )
                    pending_head = [make_head(c, h3, tb) for tb in range(n // P)]
                else:
                    # pipeline the final head into L2: after m's ELU, emit
                    # head k-tile m-2 for both 128-token blocks (ELU of
                    # k-tile m-2 is safely complete by then), so only k=6,7
                    # trail the last ELU.
                    ntb = n // P  # 2
                    pvs = [
                        psVpool.tile([P, 1], F32, name="pv", tag="pv")
                        for _ in range(ntb)
                    ]

                    def last_head_k(k):
                        def emit():
                            for tb in range(ntb):
                                nc.tensor.matmul(
                                    pvs[tb][:],
                                    lhsT=h3[:, k, tb * P : tb * P + P],
                                    rhs=wosb[:, k : k + 1],
                                    start=(k == 0),
                                    stop=(k == KH - 1),
                                    skip_group_check=True,
                                )

                        return emit

                    layer(
                        lambda k, m: w2sb[:, k, m * P : (m + 1) * P],
                        KH,
                        lambda k: h2[:, k, 0:n],
                        2 * MH,
                        h3,
                        n,
                        after_m=[None, None] + [last_head_k(k) for k in range(KH - 2)],
                    )
                    last_head_k(KH - 2)()
                    last_head_k(KH - 1)()
                    # t=16 -> reversed column 0; blk == tb here (g = 32+tb)
                    for tb in range(ntb):
                        nc.scalar.activation(
                            valT[tb][:, 0:1], pvs[tb][:], ACTF.Identity, bias=bosb[:]
                        )

            # ---- GAE epilogue: only column 0 work trails the final head ----
            for blk in range(2):
                dtt = dtts[blk]
                nc.vector.scalar_tensor_tensor(
                    dtt[:, 0:1],
                    contsb[blk][:, 0:1],
                    DISCOUNT,
                    valT[blk][:, 0:1],
                    ALU.mult,
                    ALU.mult,
                )
                nc.vector.tensor_add(dtt[:, 0:1], dtt[:, 0:1], rewsb[blk][:, 0:1])
                nc.vector.tensor_sub(dtt[:, 0:1], dtt[:, 0:1], valT[blk][:, 1:2])
                adv = gaepool.tile([P, T], F32, name=f"adv{blk}", tag=f"adv{blk}")
                nc.vector.tensor_tensor_scan(
                    adv[:], dls[blk][:], dtt[:], 0.0, ALU.mult, ALU.add
                )
                ret = gaepool.tile([P, T], F32, name=f"ret{blk}", tag=f"ret{blk}")
                nc.vector.tensor_add(ret[:], adv[:], valT[blk][:, 1:TP1])
                nc.sync.dma_start(out=ret_h[blk * P : (blk + 1) * P, :], in_=ret[:])
                nc.gpsimd.dma_start(
                    out=val_h[blk * P : (blk + 1) * P, :], in_=valT[blk][:, 1:TP1]
                )

    nc.compile()
    return nc


def _get_nc():
    global _NC_CACHE
    if _NC_CACHE is None:
        _NC_CACHE = _build()
    return _NC_CACHE


def _make_in_maps(inputs):
    import ml_dtypes

    bf16 = ml_dtypes.bfloat16
    states = np.asarray(inputs["states"], dtype=np.float32)
    reward = np.asarray(inputs["reward"], dtype=np.float32)
    cont = np.asarray(inputs["cont"], dtype=np.float32)

    # [17, B, D] -> bf16 -> [D, 17, B] -> per-core [p, k, tok]
    ST = np.ascontiguousarray(states.astype(bf16).transpose(2, 0, 1))

    # [D, H] -> [m, p, k, q]: one efficient DMA per output-column block m
    W0 = np.ascontiguousarray(
        np.asarray(inputs["W0"], np.float32)
        .astype(bf16)
        .reshape(KD, P, MH, P)
        .transpose(2, 1, 0, 3)
        .reshape(MH * P, KD * P)
    )
    # [H, H] -> [p, k, col]
    W1 = np.ascontiguousarray(
        np.asarray(inputs["W1"], np.float32)
        .astype(bf16)
        .reshape(KH, P, H)
        .transpose(1, 0, 2)
    )
    W2 = np.ascontiguousarray(
        np.asarray(inputs["W2"], np.float32)
        .astype(bf16)
        .reshape(KH, P, H)
        .transpose(1, 0, 2)
    )
    Wo = np.ascontiguousarray(
        np.asarray(inputs["Wo"], np.float32).reshape(KH, P).T.astype(bf16)
    )
    biases = np.ascontiguousarray(
        np.concatenate(
            [
                np.asarray(inputs[b], np.float32).reshape(MH, P).T
                for b in ("b0", "b1", "b2")
            ],
            axis=1,
        )
    )
    bo = np.ascontiguousarray(
        np.broadcast_to(np.asarray(inputs["bo"], np.float32).reshape(1, 1), (P, 1))
    )

    in_maps = []
    for c in range(NCORES):
        sl = slice(c * BC, (c + 1) * BC)
        stc = (
            np.ascontiguousarray(ST[:, :, sl])
            .reshape(D, TOK)
            .reshape(KD, P, TOK)
            .transpose(1, 0, 2)
        )
        gae_in = np.concatenate(
            [
                np.ascontiguousarray(cont[::-1, sl].T),
                np.ascontiguousarray(reward[::-1, sl].T),
            ],
            axis=1,
        )
        in_maps.append(
            {
                "statesP": np.ascontiguousarray(stc),
                "gae_in": np.ascontiguousarray(gae_in),
                "W0": W0,
                "W1": W1,
                "W2": W2,
                "Wo": Wo,
                "biases": biases,
                "bo_b": bo,
            }
        )
    return in_maps


def _run(inputs, trace=False):
    try:
        import profhook

        profhook.ensure_hook()
    except Exception:
        pass
    from concourse.bass_utils import run_bass_kernel_spmd

    nc = _get_nc()
    in_maps = _make_in_maps(inputs)
    bkr = run_bass_kernel_spmd(nc, in_maps, list(range(NCORES)), trace=trace)
    ret = np.empty((T, B), np.float32)
    val = np.empty((T, B), np.float32)
    for c in range(NCORES):
        sl = slice(c * BC, (c + 1) * BC)
        ret[:, sl] = bkr.results[c]["ret_bt"].T[::-1]
        val[:, sl] = bkr.results[c]["val_bt"].T[::-1]
    return (ret, val), bkr


def kernel(**inputs):
    out, _ = _run(inputs, trace=False)
    return out


# revision 6
# speedup vs baseline: 1.0171x; 1.0171x over previous
"""Trainium2 Bass kernel for nn_Critic (MLP value function + GAE).

Sharding: batch B=2048 split across 8 NeuronCores (256 each). MLP params
replicated. The time recurrence (reverse GAE scan) is independent per batch
element, so no cross-core communication.

Strategy (single-pass bf16; PE streaming floor ~464 us/core):
  - Host pre-transposes states to [P, KD, T+1 * BC] bf16 per core, so the PE
    does zero transposes; DMA loads feature-major k-tiles directly.
  - Tokens (t, b) are flattened: 17*256 = 4352 tokens per core, processed
    in chunks of 512 (one fp32 PSUM bank). All matmuls single-pass bf16
    (1 col/cycle @2.4GHz warm): end-to-end max relerr ~5e-3 vs the 2e-2 gate.
  - Startup (the trace showed first MATMUL at 14.5us with DMAs issued
    serially on Sync at ~700ns each): chunk-0 state tiles + W0 pieces are
    issued from FOUR engines in parallel (sync/scalar/vector/gpsimd), W0's
    m=0 block is split so the first LDWEIGHTS only waits on a 128KB slice,
    and 8 dummy matmuls on a memset tile warm the PE HAM clock (cold 1.2GHz
    -> warm 2.4GHz needs ~3.4us of busy) while a dummy Exp preloads the
    ScalarE activation table (~2.7us) before the first real ELU.
  - Steady state: ONE 3D-AP DMA per chunk loads all 16 k-tiles of states
    (144 -> 8 dma_starts; each dma_start costs ~700ns of issue and the
    kernel-tail semaphore teardown scales with sync traffic). W1/W2/biases/
    GAE inputs are single consolidated DMAs.
  - ELU(z) = min(exp(z)-1, relu(z)): ScalarE Exp + ScalarE Relu (both with
    fused +bias from PSUM), one VectorE combine writing bf16 directly.
  - value head: h3 (bf16) stationary [128 h, 128 tokens], Wo column moving
    -> psum [128 tokens, 1] accumulated over 8 k-tiles. Head matmuls for
    chunk c are deferred into chunk c+1's layer-0 stream so the PE never
    waits on the last ELU at a chunk boundary. ScalarE Identity with fused
    +bo writes valT [128 batch, 17 time] (stored time-reversed). The LAST
    chunk's head is pipelined into its own L2 m-loop (head k-tile m-2
    after m's ELU) so only k=6,7 trail the final ELU.
  - GAE: a handful of [128, 16/17] VectorE ops; the reverse scan is a
    single tensor_tensor_scan (state = dl*state + delta) since the host
    pre-reverses reward/cont and valT is written reversed. delta columns
    1..15 are precomputed before the final head lands so only column 0
    (t=16) trails it.
"""

import sys

sys.path.insert(0, "/opt/trn_rl_repo")

import numpy as np

T, B, D, H = 16, 2048, 2048, 1024
NCORES = 8
BC = B // NCORES  # 256 batch per core
TP1 = T + 1
TOK = TP1 * BC  # 4352 tokens per core
DISCOUNT, LAMBDA = 0.99, 0.95
P = 128
KD = D // P  # 16 k-tiles for layer 0
KH = H // P  # 8 k-tiles for layers 1,2,out
MH = H // P  # 8 m-tiles of hidden units
CH = 512  # tokens per chunk (one PSUM bank of fp32)
NCH = (TOK + CH - 1) // CH  # 9 chunks: 8 full + 1 of 256
NWARM = 12  # dummy matmuls to warm the PE HAM clock gate (~5us contiguous)

_NC_CACHE = None


def _build():
    import concourse.bacc as bacc
    import concourse.mybir as mybir
    from concourse.tile import TileContext

    F32 = mybir.dt.float32
    BF16 = mybir.dt.bfloat16
    ALU = mybir.AluOpType
    ACTF = mybir.ActivationFunctionType

    nc = bacc.Bacc(None, target_bir_lowering=False, debug=False)

    # states as [p, k, tok]: one 3D-AP DMA per chunk covers all 16 k-tiles
    st_h = nc.declare_dram_parameter("statesP", [P, KD, TOK], BF16, isOutput=False)
    # W0 host-reordered m-major: row m*P+p, col k*P+q  <-  W0[k*P+p, m*P+q],
    # so each output-column block m loads with ONE efficient DMA (4KB rows)
    w0_h = nc.declare_dram_parameter("W0", [MH * P, KD * P], BF16, isOutput=False)
    # W1/W2 as [p, k, col]: one DMA each
    w1_h = nc.declare_dram_parameter("W1", [P, KH, H], BF16, isOutput=False)
    w2_h = nc.declare_dram_parameter("W2", [P, KH, H], BF16, isOutput=False)
    wo_h = nc.declare_dram_parameter("Wo", [P, KH], BF16, isOutput=False)
    bo_h = nc.declare_dram_parameter("bo_b", [P, 1], F32, isOutput=False)
    # b0|b1|b2 packed: one DMA
    bias_h = nc.declare_dram_parameter("biases", [P, 3 * MH], F32, isOutput=False)
    # cont_rev|rew_rev packed per 128-batch block: one DMA each
    gae_h = nc.declare_dram_parameter("gae_in", [BC, TP1 + T], F32, isOutput=False)
    ret_h = nc.declare_dram_parameter("ret_bt", [BC, T], F32, isOutput=True)
    val_h = nc.declare_dram_parameter("val_bt", [BC, T], F32, isOutput=True)

    with TileContext(nc) as tc:
        with (
            tc.tile_pool(name="warm", bufs=1) as warmpool,
            tc.tile_pool(name="wpool", bufs=1) as wpool,
            tc.tile_pool(name="x0pool", bufs=1) as x0pool,
            tc.tile_pool(name="xpool", bufs=2) as xpool,
            tc.tile_pool(name="hpool", bufs=2) as hpool,
            tc.tile_pool(name="tmp", bufs=4) as tmppool,
            tc.tile_pool(name="gae", bufs=1) as gaepool,
            tc.tile_pool(name="psA", bufs=6, space="PSUM") as psApool,
            tc.tile_pool(name="psV", bufs=2, space="PSUM") as psVpool,
        ):
            # ---- PE/ACT warmup: memset tile -> 8 dummy matmuls + dummy Exp.
            # No DMA deps, so these issue immediately (~6.3us, engine bring-up
            # done) and run while the first input DMAs are in flight; the HAM
            # clock gate un-throttles after ~3.4us of sustained PE activity.
            warm = warmpool.tile([P, CH], BF16, name="warm", tag="warm")
            nc.gpsimd.memset(warm[:], 0.0)
            wtmp = warmpool.tile([P, 1], F32, name="wtmp", tag="wtmp")
            for _ in range(NWARM):
                psd = psApool.tile([P, CH], F32, name="ps", tag="ps")
                nc.tensor.matmul(
                    psd[:],
                    lhsT=warm[:, 0:P],
                    rhs=warm[:],
                    start=True,
                    stop=True,
                    skip_group_check=True,
                )

            # ---- weight / input tiles ----
            w0m0a = wpool.tile([P, 4 * P], BF16, name="w0m0a", tag="w0m0a")
            w0m0b = wpool.tile([P, (KD - 4) * P], BF16, name="w0m0b", tag="w0m0b")
            w0m = [None] + [
                wpool.tile([P, KD * P], BF16, name=f"w0m{m}", tag=f"w0m{m}")
                for m in range(1, MH)
            ]
            w1sb = wpool.tile([P, KH, H], BF16, name="w1sb", tag="w1sb")
            w2sb = wpool.tile([P, KH, H], BF16, name="w2sb", tag="w2sb")
            wosb = wpool.tile([P, KH], BF16, name="wosb", tag="wosb")
            bosb = wpool.tile([P, 1], F32, name="bosb", tag="bosb")
            biassb = wpool.tile([P, 3 * MH], F32, name="biassb", tag="biassb")
            xts0 = [
                x0pool.tile([P, CH], BF16, name=f"xt{k}", tag=f"xt{k}")
                for k in range(KD)
            ]
            gaesb = []
            valT = []
            for blk in range(2):
                g = gaepool.tile(
                    [P, TP1 + T], F32, name=f"gaesb{blk}", tag=f"gaesb{blk}"
                )
                gaesb.append(g)
                vt = gaepool.tile([P, TP1], F32, name=f"valT{blk}", tag=f"valT{blk}")
                valT.append(vt)
            contsb = [g[:, 0:TP1] for g in gaesb]
            rewsb = [g[:, TP1 : TP1 + T] for g in gaesb]

            # ---- startup DMAs, issued from THREE engines in parallel (only
            # sync/scalar/gpsimd can trigger DMAs). Per-engine program order
            # = issue order AND ring-FIFO transfer order — each engine's
            # HWDGE ring processes its transfers strictly in order, which is
            # the priority mechanism: later transfers on the same ring can't
            # steal bandwidth from earlier ones. Each dma_start costs ~700ns
            # of that engine's instruction time.
            # scalar ring: first-MM weight deps, then the dummy-Exp ACT-table
            # preload (AFTER the critical issues — TABLE_LOAD holds the
            # Scalar queue ~2.7us), then head weights.
            nc.scalar.dma_start(out=w0m0a[:], in_=w0_h[0:P, 0 : 4 * P])
            nc.scalar.dma_start(out=w0m0b[:], in_=w0_h[0:P, 4 * P : KD * P])
            nc.scalar.dma_start(out=w0m[1][:], in_=w0_h[P : 2 * P, :])
            nc.scalar.dma_start(out=biassb[:], in_=bias_h[:])
            nc.scalar.activation(wtmp[:], warm[:, 0:1], ACTF.Exp)
            nc.scalar.dma_start(out=wosb[:], in_=wo_h[:])
            # sync / gpsimd rings: chunk-0 state k-tiles round-robin first,
            # then the remaining weights BEHIND them (ring-FIFO keeps the
            # 4MB of W0 m>=2 / W1 / W2 from racing chunk-0's states).
            for k in range(KD):
                eng = (nc.sync, nc.gpsimd)[k % 2]
                eng.dma_start(out=xts0[k][:], in_=st_h[:, k, 0:CH])
            for m in range(2, MH):
                nc.sync.dma_start(out=w0m[m][:], in_=w0_h[m * P : (m + 1) * P, :])
            nc.sync.dma_start(out=w1sb[:], in_=w1_h[:])
            nc.sync.dma_start(out=w2sb[:], in_=w2_h[:])
            nc.gpsimd.dma_start(out=bosb[:], in_=bo_h[:])
            for blk in range(2):
                nc.gpsimd.dma_start(
                    out=gaesb[blk][:], in_=gae_h[blk * P : (blk + 1) * P, :]
                )

            def l0_lhsT(k, m):
                if m == 0:
                    if k < 4:
                        return w0m0a[:, k * P : (k + 1) * P]
                    return w0m0b[:, (k - 4) * P : (k - 3) * P]
                return w0m[m][:, k * P : (k + 1) * P]

            # ---- chunked fused MLP over flattened (t, b) tokens ----
            def layer(lhsT_of_km, nk, rhs_of_k, bias_col, houts, n, after_m=()):
                for m in range(MH):
                    ps = psApool.tile([P, CH], F32, name="ps", tag="ps")
                    for k in range(nk):
                        nc.tensor.matmul(
                            ps[:, :n],
                            lhsT=lhsT_of_km(k, m),
                            rhs=rhs_of_k(k),
                            start=(k == 0),
                            stop=(k == nk - 1),
                            skip_group_check=True,
                        )
                    e = tmppool.tile([P, CH], F32, name="e", tag="e")
                    nc.scalar.activation(
                        e[:, :n],
                        ps[:, :n],
                        ACTF.Exp,
                        bias=biassb[:, bias_col + m : bias_col + m + 1],
                    )
                    rl = tmppool.tile([P, CH], F32, name="rl", tag="rl")
                    nc.scalar.activation(
                        rl[:, :n],
                        ps[:, :n],
                        ACTF.Relu,
                        bias=biassb[:, bias_col + m : bias_col + m + 1],
                    )
                    nc.vector.scalar_tensor_tensor(
                        houts[:, m, 0:n],
                        e[:, :n],
                        1.0,
                        rl[:, :n],
                        ALU.subtract,
                        ALU.min,
                    )
                    if m < len(after_m) and after_m[m] is not None:
                        after_m[m]()

            # value head for one 128-token block: h3 stationary, Wo moving.
            def make_head(c, h3, tb):
                g = c * (CH // P) + tb  # global 128-token block
                t_idx = g // 2
                blk = g % 2

                def emit():
                    pv = psVpool.tile([P, 1], F32, name="pv", tag="pv")
                    for k in range(KH):
                        nc.tensor.matmul(
                            pv[:],
                            lhsT=h3[:, k, tb * P : tb * P + P],
                            rhs=wosb[:, k : k + 1],
                            start=(k == 0),
                            stop=(k == KH - 1),
                            skip_group_check=True,
                        )
                    # store time-REVERSED: column 16-t, with fused +bo
                    nc.scalar.activation(
                        valT[blk][:, TP1 - 1 - t_idx : TP1 - t_idx],
                        pv[:],
                        ACTF.Identity,
                        bias=bosb[:],
                    )

                return emit

            dls = [None, None]
            dtts = [None, None]

            pending_head = []
            for c in range(NCH):
                base = c * CH
                n = min(CH, TOK - base)
                last = c == NCH - 1
                if c == 0:
                    xts = None
                else:
                    # gpsimd ring: keeps mid-kernel state prefetch off the
                    # sync ring where the W0/W1/W2 startup queue lives.
                    xts = xpool.tile([P, KD, CH], BF16, name="xts", tag="xts")
                    nc.gpsimd.dma_start(
                        out=xts[:, :, 0:n], in_=st_h[:, :, base : base + n]
                    )

                h1 = hpool.tile([P, MH, CH], BF16, name="h1", tag="h1")
                layer(
                    l0_lhsT,
                    KD,
                    (lambda k: xts0[k][:, :n]) if c == 0 else (lambda k: xts[:, k, 0:n]),
                    0,
                    h1,
                    n,
                    after_m=pending_head,
                )
                pending_head = []

                if c == 1:
                    # dl = cont*DISCOUNT*LAMBDA for the scan; Vector is
                    # otherwise idle and the GAE inputs landed at startup.
                    for blk in range(2):
                        dl = gaepool.tile([P, T], F32, name=f"dl{blk}", tag=f"dl{blk}")
                        nc.vector.tensor_scalar_mul(
                            dl[:], contsb[blk][:, 0:T], DISCOUNT * LAMBDA
                        )
                        dls[blk] = dl
                if last:
                    # deltas for time-reversed columns 1..15 depend only on
                    # value columns 1..16, all of which landed with chunk 7's
                    # deferred head (played in this chunk's L0). Precompute
                    # them so only column 0 (t=16) trails the final head.
                    for blk in range(2):
                        dtt = gaepool.tile(
                            [P, T], F32, name=f"dtt{blk}", tag=f"dtt{blk}"
                        )
                        nc.vector.scalar_tensor_tensor(
                            dtt[:, 1:T],
                            contsb[blk][:, 1:T],
                            DISCOUNT,
                            valT[blk][:, 1:T],
                            ALU.mult,
                            ALU.mult,
                        )
                        nc.vector.tensor_add(
                            dtt[:, 1:T], dtt[:, 1:T], rewsb[blk][:, 1:T]
                        )
                        nc.vector.tensor_sub(
                            dtt[:, 1:T], dtt[:, 1:T], valT[blk][:, 2:TP1]
                        )
                        dtts[blk] = dtt

                h2 = hpool.tile([P, MH, CH], BF16, name="h2", tag="h2")
                layer(
                    lambda k, m: w1sb[:, k, m * P : (m + 1) * P],
                    KH,
                    lambda k: h1[:, k, 0:n],
                    MH,
                    h2,
                    n,
                )
                h3 = hpool.tile([P, MH, CH], BF16, name="h3", tag="h3")

                if not last:
                    layer(
                        lambda k, m: w2sb[:, k, m * P : (m + 1) * P],
                        KH,
                        lambda k: h2[:, k, 0:n],
                        2 * MH,
                        h3,
                        n,
                    )
                    pending_head = [make_head(c, h3, tb) for tb in range(n // P)]
                else:
                    # pipeline the final head into L2: after m's ELU, emit
                    # head k-tile m-2 for both 128-token blocks (ELU of
                    # k-tile m-2 is safely complete by then), so only k=6,7
                    # trail the last ELU.
                    ntb = n // P  # 2
                    pvs = [
                        psVpool.tile([P, 1], F32, name="pv", tag="pv")
                        for _ in range(ntb)
                    ]

                    def last_head_k(k):
                        def emit():
                            for tb in range(ntb):
                                nc.tensor.matmul(
                                    pvs[tb][:],
                                    lhsT=h3[:, k, tb * P : tb * P + P],
                                    rhs=wosb[:, k : k + 1],
                                    start=(k == 0),
                                    stop=(k == KH - 1),
                                    skip_group_check=True,
                                )

                        return emit

                    layer(
                        lambda k, m: w2sb[:, k, m * P : (m + 1) * P],
                        KH,
                        lambda k: h2[:, k, 0:n],
                        2 * MH,
                        h3,
                        n,
                        after_m=[None, None] + [last_head_k(k) for k in range(KH - 2)],
                    )
                    last_head_k(KH - 2)()
                    last_head_k(KH - 1)()
                    # t=16 -> reversed column 0; blk == tb here (g = 32+tb)
                    for tb in range(ntb):
                        nc.scalar.activation(
                            valT[tb][:, 0:1], pvs[tb][:], ACTF.Identity, bias=bosb[:]
                        )

            # ---- GAE epilogue: only column 0 work trails the final head ----
            for blk in range(2):
                dtt = dtts[blk]
                nc.vector.scalar_tensor_tensor(
                    dtt[:, 0:1],
                    contsb[blk][:, 0:1],
                    DISCOUNT,
                    valT[blk][:, 0:1],
                    ALU.mult,
                    ALU.mult,
                )
                nc.vector.tensor_add(dtt[:, 0:1], dtt[:, 0:1], rewsb[blk][:, 0:1])
                nc.vector.tensor_sub(dtt[:, 0:1], dtt[:, 0:1], valT[blk][:, 1:2])
                adv = gaepool.tile([P, T], F32, name=f"adv{blk}", tag=f"adv{blk}")
                nc.vector.tensor_tensor_scan(
                    adv[:], dls[blk][:], dtt[:], 0.0, ALU.mult, ALU.add
                )
                ret = gaepool.tile([P, T], F32, name=f"ret{blk}", tag=f"ret{blk}")
                nc.vector.tensor_add(ret[:], adv[:], valT[blk][:, 1:TP1])
                nc.sync.dma_start(out=ret_h[blk * P : (blk + 1) * P, :], in_=ret[:])
                nc.gpsimd.dma_start(
                    out=val_h[blk * P : (blk + 1) * P, :], in_=valT[blk][:, 1:TP1]
                )

    nc.compile()
    return nc


def _get_nc():
    global _NC_CACHE
    if _NC_CACHE is None:
        _NC_CACHE = _build()
    return _NC_CACHE


def _make_in_maps(inputs):
    import ml_dtypes

    bf16 = ml_dtypes.bfloat16
    states = np.asarray(inputs["states"], dtype=np.float32)
    reward = np.asarray(inputs["reward"], dtype=np.float32)
    cont = np.asarray(inputs["cont"], dtype=np.float32)

    # [17, B, D] -> bf16 -> [D, 17, B] -> per-core [p, k, tok]
    ST = np.ascontiguousarray(states.astype(bf16).transpose(2, 0, 1))

    # [D, H] -> [m, p, k, q]: one efficient DMA per output-column block m
    W0 = np.ascontiguousarray(
        np.asarray(inputs["W0"], np.float32)
        .astype(bf16)
        .reshape(KD, P, MH, P)
        .transpose(2, 1, 0, 3)
        .reshape(MH * P, KD * P)
    )
    # [H, H] -> [p, k, col]
    W1 = np.ascontiguousarray(
        np.asarray(inputs["W1"], np.float32)
        .astype(bf16)
        .reshape(KH, P, H)
        .transpose(1, 0, 2)
    )
    W2 = np.ascontiguousarray(
        np.asarray(inputs["W2"], np.float32)
        .astype(bf16)
        .reshape(KH, P, H)
        .transpose(1, 0, 2)
    )
    Wo = np.ascontiguousarray(
        np.asarray(inputs["Wo"], np.float32).reshape(KH, P).T.astype(bf16)
    )
    biases = np.ascontiguousarray(
        np.concatenate(
            [
                np.asarray(inputs[b], np.float32).reshape(MH, P).T
                for b in ("b0", "b1", "b2")
            ],
            axis=1,
        )
    )
    bo = np.ascontiguousarray(
        np.broadcast_to(np.asarray(inputs["bo"], np.float32).reshape(1, 1), (P, 1))
    )

    in_maps = []
    for c in range(NCORES):
        sl = slice(c * BC, (c + 1) * BC)
        stc = (
            np.ascontiguousarray(ST[:, :, sl])
            .reshape(D, TOK)
            .reshape(KD, P, TOK)
            .transpose(1, 0, 2)
        )
        gae_in = np.concatenate(
            [
                np.ascontiguousarray(cont[::-1, sl].T),
                np.ascontiguousarray(reward[::-1, sl].T),
            ],
            axis=1,
        )
        in_maps.append(
            {
                "statesP": np.ascontiguousarray(stc),
                "gae_in": np.ascontiguousarray(gae_in),
                "W0": W0,
                "W1": W1,
                "W2": W2,
                "Wo": Wo,
                "biases": biases,
                "bo_b": bo,
            }
        )
    return in_maps


def _run(inputs, trace=False):
    try:
        import profhook

        profhook.ensure_hook()
    except Exception:
        pass
    from concourse.bass_utils import run_bass_kernel_spmd

    nc = _get_nc()
    in_maps = _make_in_maps(inputs)
    bkr = run_bass_kernel_spmd(nc, in_maps, list(range(NCORES)), trace=trace)
    ret = np.empty((T, B), np.float32)
    val = np.empty((T, B), np.float32)
    for c in range(NCORES):
        sl = slice(c * BC, (c + 1) * BC)
        ret[:, sl] = bkr.results[c]["ret_bt"].T[::-1]
        val[:, sl] = bkr.results[c]["val_bt"].T[::-1]
    return (ret, val), bkr


def kernel(**inputs):
    out, _ = _run(inputs, trace=False)
    return out
